# revision 27
# baseline (speedup 1.0000x reference)
"""Trainium2 Bass kernel v7 for nn_Attention_39651138076722.

ChannelLayerNorm -> qkv 1x1 conv -> 4-head spatial attention (N=4096, dh=32)
-> proj 1x1 conv -> residual.   B=4, C=128, H=W=64.
8 cores = 4 batches x 2 head-pairs; host sums the two partials per batch.

Phase 1 (LN + qkv) runs in its own 8-slot 1-bank PSUM ring (depth-8 pipeline
instead of depth-3), closed before the attention pools open. Phase 2 per
n-chunk j of 512 positions: 32 groups, each = both heads of one m-chunk.
S^T = kk2^T qq2 in bf16 (2 row-packed matmuls into a 2-bank PSUM tile from a
3-pool rotation), exp on ACT (native) or DVE (int16 Schraudolph bit-trick ->
bf16), PV accumulation in bf16 into one shared PSUM bank (heads col-packed at
partition 0/64), deferred six entries so the PE queue never blocks on an exp;
chunk-j PV leftovers drain during chunk j+1's first supers. Denominators sit
at pv rows 32/96; one eps-biased Ln over pv[0:97] + one exp gives both
reciprocals; a 96-row broadcast DMA and one merged TT apply them (rows 33-63
are a zero band, killed by zero rows in the 97-row proj weight).
"""
import sys
sys.path.insert(0, "/opt/trn_rl_repo")

import numpy as np
import concourse.bass as bass
import concourse.tile as tile
from concourse import bacc, mybir
from concourse.bass_utils import run_bass_kernel_spmd

# Pin every ACT function to the natural_log_exp_and_others table set so the
# table-load pass never alternates sets (each switch costs ~2.7us on ACT).
_orig_get_tables = bacc.get_activation_tables


def _pinned_tables(arch):
    tabs = _orig_get_tables(arch)
    keep = "natural_log_exp_and_others"
    pinned = tabs[keep]
    return {n: (f if n == keep else f - pinned) for n, f in tabs.items()}


bacc.get_activation_tables = _pinned_tables

F32 = mybir.dt.float32
F32R = mybir.dt.float32r
I16 = mybir.dt.int16
BF16 = mybir.dt.bfloat16
AF = mybir.ActivationFunctionType
OP = mybir.AluOpType

B, C, H, W = 4, 128, 64, 64
N = H * W                      # 4096
NH, DH = 4, 32
EPS = 1e-6
NCH = 512                      # free-dim chunk (psum bank)
NJ = N // NCH                  # 8 n-chunks
MC = 128                       # m-chunk (partition tile)
NM = N // MC                   # 32 m-chunks
SCALE = DH ** -0.5

# Schraudolph exp constants (bf16, truncation toward zero)
EXP_A = float(2.0 ** 7 / np.log(2.0))
EXP_B = float(127.0 * 2 ** 7 - 5.59)

# mostly-strict A/D alternation (one ACT + one DVE exp per super-group).
# 18 ACT / 14 DVE; the supers at gs8/gs12 run A,A: while DVE chews the tail
# ops (hn mul, ot stt) issued right after gs8/gs12, the S psum pools recycle
# through ACT instead of waiting on DVE's backed-up FIFO.
ENG = ['A' if g % 2 == 0 else 'D' for g in range(32)]
ENG[9] = 'A'
ENG[13] = 'A'              # 18 ACT / 14 DVE
N_FILL = 1
FILL_N = 256


def build_nc():
    nc = bacc.Bacc("TRN2", target_bir_lowering=False)
    d_x = nc.dram_tensor("x", [C, N], F32, kind="ExternalInput")
    d_wqq = nc.dram_tensor("wqq", [C, 128], F32, kind="ExternalInput")
    d_wkk = nc.dram_tensor("wkk", [C, 128], F32, kind="ExternalInput")
    d_wv = nc.dram_tensor("wv", [C, 64], F32, kind="ExternalInput")
    d_bqq = nc.dram_tensor("bqq", [128, 1], F32, kind="ExternalInput")
    d_bkk = nc.dram_tensor("bkk", [128, 1], F32, kind="ExternalInput")
    d_bv = nc.dram_tensor("bv", [C, 4, 64], F32, kind="ExternalInput")
    d_pw = nc.dram_tensor("pw", [97, C], F32, kind="ExternalInput")
    d_res = nc.dram_tensor("res", [C, 1], F32, kind="ExternalInput")
    d_out = nc.dram_tensor("out", [C, N], F32, kind="ExternalOutput")

    pairs = [(i % 2, i // 2) for i in range(2 * NM)]   # (head, m-chunk)

    with tile.TileContext(nc) as tc:
        with tc.tile_pool(name="persist", bufs=1) as P:
            x_sb = P.tile([C, N], F32, tag="x_sb")
            xhat = P.tile([C, N], BF16, tag="xhat")
            qq2 = P.tile([C, N], BF16, tag="qq2")
            kk2 = P.tile([C, N], BF16, tag="kk2")
            vta = P.tile([C, NM, 66], BF16, tag="vta")   # [v0|1|v1|1] per mc
            w_qq = P.tile([C, 128], F32, tag="w_qq")
            w_kk = P.tile([C, 128], F32, tag="w_kk")
            w_v = P.tile([C, 64], F32, tag="w_v")
            wb_qq = P.tile([C, 128], BF16, tag="wb_qq")
            wb_kk = P.tile([C, 128], BF16, tag="wb_kk")
            wb_v = P.tile([C, 64], BF16, tag="wb_v")
            b_qq = P.tile([128, 1], F32, tag="b_qq")
            b_kk = P.tile([128, 1], F32, tag="b_kk")
            bv_b = P.tile([C, 4, 64], F32, tag="bv_b")
            w_p = P.tile([97, C], F32, tag="w_p")
            wb_p = P.tile([97, C], BF16, tag="wb_p")
            res_c = P.tile([C, 1], F32, tag="res_c")
            ones_m = P.tile([C, C], F32, tag="ones_m")
            onesC_m = P.tile([C, C], F32, tag="onesC_m")     # 1/C
            onesC_r = P.tile([C, C], F32R, tag="onesC_r")
            zer33 = P.tile([C, 33], BF16, tag="zer33")
            zrhs = P.tile([C, NCH], BF16, tag="zrhs")
            eps_c = P.tile([C, 1], F32, tag="eps_c")
            ones_n = P.tile([1, NCH], F32, tag="ones_n")
            # normalized h (rows 0-31 h0, 64-95 h1, 32-63 zero band) + ones@96
            hn_a = P.tile([97, NCH], BF16, tag="hn_a")
            hn_b = P.tile([97, NCH], BF16, tag="hn_b")

            # x chunks on the sync queue FIRST (the LN chain gates phase 1);
            # weights ride the idle gpsimd queue so the two issue in parallel
            JS = [slice(j * NCH, (j + 1) * NCH) for j in range(NJ)]
            for j in range(NJ):
                nc.sync.dma_start(out=x_sb[:, JS[j]], in_=d_x.ap()[:, JS[j]])
            nc.gpsimd.dma_start(out=w_qq, in_=d_wqq.ap())
            nc.gpsimd.dma_start(out=w_kk, in_=d_wkk.ap())
            nc.gpsimd.dma_start(out=w_v, in_=d_wv.ap())
            nc.gpsimd.dma_start(out=b_qq, in_=d_bqq.ap())
            nc.gpsimd.dma_start(out=b_kk, in_=d_bkk.ap())
            nc.gpsimd.dma_start(out=bv_b, in_=d_bv.ap())
            nc.gpsimd.dma_start(out=w_p, in_=d_pw.ap())
            nc.gpsimd.dma_start(out=res_c, in_=d_res.ap())
            nc.vector.memset(ones_m, 1.0)
            nc.vector.memset(onesC_m, 1.0 / C)
            nc.vector.memset(zer33, 0.0)
            nc.vector.memset(zrhs, 0.0)
            nc.vector.memset(eps_c, EPS)
            nc.vector.memset(ones_n, 1.0)
            nc.vector.tensor_copy(out=onesC_r, in_=onesC_m)
            nc.vector.tensor_copy(out=wb_qq, in_=w_qq)
            nc.vector.tensor_copy(out=wb_kk, in_=w_kk)
            nc.vector.tensor_copy(out=wb_v, in_=w_v)
            nc.vector.tensor_copy(out=wb_p, in_=w_p)
            nc.vector.tensor_copy(out=hn_a[96:97, :], in_=ones_n)
            nc.vector.tensor_copy(out=hn_b[96:97, :], in_=ones_n)
            nc.vector.tensor_copy(out=vta[:, :, 32:33], in_=ones_m[:, 0:NM])
            nc.vector.tensor_copy(out=vta[:, :, 65:66], in_=ones_m[:, 0:NM])

            with tc.tile_pool(name="stats", bufs=4) as SP, \
                 tc.tile_pool(name="dscr", bufs=1, space="DRAM") as DSC, \
                 tc.tile_pool(name="pta", bufs=5) as PTA, \
                 tc.tile_pool(name="ptd", bufs=8) as PTD, \
                 tc.tile_pool(name="rbp", bufs=2) as RBP, \
                 tc.tile_pool(name="opool", bufs=2) as OPO:
                scr = [DSC.tile([3, NCH], F32, tag=f"scr{j}", name=f"scr{j}")
                       for j in range(NJ)]

                # ---------- phase 1: LN + qkv in a depth-8 1-bank ring ------
                with tc.tile_pool(name="mscp", bufs=8, space="PSUM") as MSCP:
                    def P1_tile(shape, name):
                        return MSCP.tile(shape, F32, tag="msc", name=name)

                    x2s, s1s, s2s, msqs, cens, invs = {}, {}, {}, {}, {}, {}
                    for j in range(NJ):
                        x2s[j] = SP.tile([C, NCH], F32R, tag="x2",
                                         name=f"x2_{j}")
                        nc.vector.tensor_mul(x2s[j], x_sb[:, JS[j]],
                                             x_sb[:, JS[j]])
                        s1s[j] = P1_tile([C, NCH], f"s1_{j}")
                        nc.tensor.matmul(s1s[j], onesC_m, x_sb[:, JS[j]],
                                         start=True, stop=True)
                        s2s[j] = P1_tile([C, NCH], f"s2_{j}")
                        nc.tensor.matmul(s2s[j], onesC_r, x2s[j],
                                         start=True, stop=True)
                        msqs[j] = SP.tile([C, NCH], F32, tag="msq",
                                          name=f"msq_{j}")
                        nc.scalar.activation(out=msqs[j], in_=s1s[j],
                                             func=AF.Square, scale=1.0)
                        cens[j] = SP.tile([C, NCH], F32, tag="cen",
                                          name=f"cen_{j}")
                        nc.vector.tensor_sub(cens[j], x_sb[:, JS[j]], s1s[j])
                    for j in range(NJ):
                        var = SP.tile([C, NCH], F32, tag="var", name=f"var_{j}")
                        nc.vector.scalar_tensor_tensor(out=var, in0=s2s[j],
                                                       scalar=1.0,
                                                       in1=msqs[j],
                                                       op0=OP.mult,
                                                       op1=OP.subtract)
                        lnv = SP.tile([C, NCH], F32, tag="lnv", name=f"lnv_{j}")
                        nc.scalar.activation(out=lnv, in_=var, func=AF.Ln,
                                             bias=eps_c, scale=1.0)
                        invs[j] = SP.tile([C, NCH], F32, tag="inv",
                                          name=f"inv_{j}")
                        nc.scalar.activation(out=invs[j], in_=lnv, func=AF.Exp,
                                             scale=-0.5)
                        # xhat mul on GPSIMD (SBUF-only op): frees DVE
                        nc.gpsimd.tensor_mul(xhat[:, JS[j]], cens[j], invs[j])
                    for j in range(NJ):
                        qk = P1_tile([C, NCH], f"qk_{j}")
                        nc.tensor.matmul(qk, wb_qq, xhat[:, JS[j]],
                                         start=True, stop=True)
                        nc.scalar.activation(out=qq2[:, JS[j]], in_=qk,
                                             func=AF.Identity, bias=b_qq,
                                             scale=1.0)
                        kk = P1_tile([C, NCH], f"kk_{j}")
                        nc.tensor.matmul(kk, wb_kk, xhat[:, JS[j]],
                                         start=True, stop=True)
                        nc.scalar.activation(out=kk2[:, JS[j]], in_=kk,
                                             func=AF.Identity, bias=b_kk,
                                             scale=1.0)
                        vpq = P1_tile([C, 4, 64], f"vpq{j}")
                        for mq in range(4):
                            mc = 4 * j + mq
                            ms = slice(mc * MC, (mc + 1) * MC)
                            nc.tensor.matmul(vpq[:, mq, :], xhat[:, ms], wb_v,
                                             start=True, stop=True)
                        vdst = vta[:, 4 * j:4 * j + 4, 0:66].rearrange(
                            "p m (a b) -> p m a b", a=2)[:, :, :, 0:32]
                        vsrc = vpq.rearrange("p m (a b) -> p m a b", a=2)
                        bsrc = bv_b.rearrange("p m (a b) -> p m a b", a=2)
                        nc.vector.tensor_add(vdst, vsrc, bsrc)

                # ---------- phase 2: attention over all chunks --------------
                with tc.tile_pool(name="sp0", bufs=1, space="PSUM") as SP0, \
                     tc.tile_pool(name="sp1", bufs=1, space="PSUM") as SP1, \
                     tc.tile_pool(name="sp2", bufs=1, space="PSUM") as SP2, \
                     tc.tile_pool(name="pvpool", bufs=2, space="PSUM") as PVP:
                    SPOOLS = [SP0, SP1, SP2]
                    _msc_rr = [0]

                    def MSC_tile(shape, dtype, name):
                        pool = SPOOLS[_msc_rr[0] % 3]
                        _msc_rr[0] += 1
                        return pool.tile(shape, dtype, tag="sg", name=name)
                    pvs = {}
                    # separate deferral FIFOs per exp engine: A-group PVs
                    # drain promptly; D-group PVs lag ~6 supers so a late
                    # DVE exp never sits at the PE queue head blocking it
                    pending_a = []
                    pending_d = []

                    _fill_id = [0]

                    def ring_filler(pool, fn=FILL_N):
                        # real matmul into a throwaway ring tile: HAM only
                        # counts matmul streaming as PE activity, so fillers
                        # must be matmuls.
                        _fill_id[0] += 1
                        ft = pool.tile([33, NCH], F32, tag="sg",
                                       name=f"fill{_fill_id[0]}")
                        nc.tensor.matmul(ft[:, 0:fn], zer33, zrhs[:, 0:fn],
                                         start=True, stop=True,
                                         skip_group_check=True)

                    A_DEPTH = 6
                    D_DEPTH = 6

                    def flush_pv(force=True):
                        la = 0 if force else A_DEPTH
                        ld = 0 if force else D_DEPTH
                        while len(pending_a) > la:
                            pending_a.pop(0)[1]()
                        while len(pending_d) > ld:
                            pending_d.pop(0)[1]()

                    def flush_upto(j):
                        # drain every deferred PV belonging to chunk <= j
                        while pending_a and pending_a[0][0] <= j:
                            pending_a.pop(0)[1]()
                        while pending_d and pending_d[0][0] <= j:
                            pending_d.pop(0)[1]()

                    def new_pv(j):
                        pv = PVP.tile([128, NCH], F32, tag="pv", name=f"pv{j}")
                        pvs[j] = pv
                        return pv

                    def issue_S(j, gi, pool):
                        js = slice(j * NCH, (j + 1) * NCH)
                        g0 = 2 * gi
                        grp = pairs[g0:g0 + 2]
                        sg = pool.tile([C, 2 * NCH], F32, tag="sg",
                                       name=f"sg{j}_{gi}")
                        for i, (h, mc) in enumerate(grp):
                            rg = h + 2 * (mc % 2)
                            ms = slice(mc * MC, (mc + 1) * MC)
                            rs = slice(rg * 32, (rg + 1) * 32)
                            nc.tensor.matmul(sg[:, i * NCH:(i + 1) * NCH],
                                             kk2[rs, ms], qq2[rs, js],
                                             start=True, stop=True,
                                             tile_position=(rg * 32, 0))
                        return sg

                    def issue_exp(j, gi, sg, pool):
                        g0 = 2 * gi
                        grp = pairs[g0:g0 + 2]
                        if ENG[gi] == 'A':
                            pt = PTA.tile([C, 2 * NCH], BF16, tag="pt",
                                          name=f"pt{j}_{gi}")
                            nc.scalar.activation(out=pt, in_=sg,
                                                 func=AF.Exp, scale=SCALE)
                            ptf = pt
                        else:
                            pt = PTD.tile([C, 2 * NCH], I16, tag="pt",
                                          name=f"pt{j}_{gi}")
                            nc.vector.tensor_scalar(out=pt, in0=sg,
                                                    scalar1=EXP_A * SCALE,
                                                    scalar2=EXP_B,
                                                    op0=OP.mult, op1=OP.add)
                            ptf = pt.bitcast(BF16)
                        pv = pvs[j]

                        def do_pv():
                            if g0 == 0:
                                # pre-zero the h1 region: its chain runs
                                # flags=0 (h0's start=True clears bank)
                                nc.tensor.matmul(pv[64:97, :], zer33, zrhs,
                                                 start=True, stop=True,
                                                 tile_position=(0, 64),
                                                 skip_group_check=True)
                                # pre-zero the junk band rows 32-63 so the
                                # merged 96-row hn mul sees exact zeros there
                                # (row 32 re-cleared by h0's start=True)
                                nc.tensor.matmul(pv[32:64, :], zer33[:, 0:32],
                                                 zrhs, start=True, stop=True,
                                                 tile_position=(0, 32),
                                                 skip_group_check=True)
                            for i, (h, mc) in enumerate(grp):
                                pi = g0 + i
                                vcols = slice(33 * h, 33 * h + 33)
                                out_sl = pv[0:33, :] if h == 0 else pv[64:97, :]
                                nc.tensor.matmul(out_sl, vta[:, mc, vcols],
                                                 ptf[:, i * NCH:(i + 1) * NCH],
                                                 start=(pi == 0),
                                                 stop=(pi == 2 * NM - 2 + h),
                                                 tile_position=(
                                                     0, 0 if h == 0 else 64),
                                                 skip_group_check=True)
                        if ENG[gi] == 'A':
                            pending_a.append((j, do_pv))
                        else:
                            pending_d.append((j, do_pv))

                    def attn_super(j, gs):
                        # two groups: 4 S matmuls back-to-back (4-wide row
                        # packing), then the ACT and DVE exps run concurrently
                        pA = SPOOLS[gs % 3]
                        pD = SPOOLS[(gs + 1) % 3]
                        sgA = issue_S(j, gs, pA)
                        sgD = issue_S(j, gs + 1, pD)
                        issue_exp(j, gs, sgA, pA)
                        issue_exp(j, gs + 1, sgD, pD)
                        pending_a.append((j, lambda p=pA: ring_filler(p)))
                        flush_pv(force=False)

                    def attn_end(j):
                        # no force-flush: chunk j's last PVs drain interleaved
                        # with chunk j+1's first supers so ACT never starves
                        # at the boundary; denom_tail(j) drains what's left.
                        flush_pv(force=False)

                    def denom_tail(j):
                        # one Ln over pv[0:97] (row 32 = D_h0, row 96 = D_h1;
                        # the eps bias keeps junk rows finite), one exp.
                        flush_upto(j)
                        pv = pvs[j]
                        lnd = RBP.tile([97, NCH], F32, tag="lnd",
                                       name=f"lnd{j}")
                        rec = RBP.tile([97, NCH], F32, tag="rec",
                                       name=f"rec{j}")
                        nc.scalar.activation(out=lnd, in_=pv[0:97, :],
                                             func=AF.Ln,
                                             bias=eps_c[0:97, 0:1],
                                             scale=1.0)
                        nc.scalar.activation(out=rec, in_=lnd, func=AF.Exp,
                                             scale=-1.0)
                        nc.sync.dma_start(out=scr[j][0:2, :],
                                          in_=rec[32:34, :])
                        nc.sync.dma_start(out=scr[j][2:3, :],
                                          in_=rec[96:97, :])

                    def tail_a(j):
                        # rb rows 0-31 <- 1/D_h0, 32-63 <- finite junk (hits
                        # the zero band of pv), 64-95 <- 1/D_h1; merged mul.
                        rb = RBP.tile([96, NCH], F32, tag="rb", name=f"rb{j}")
                        src = bass.AP(tensor=scr[j].tensor,
                                      offset=scr[j].offset,
                                      ap=[[NCH, 3], [0, 32], [1, NCH]])
                        nc.sync.dma_start(out=rb, in_=src)
                        hn = hn_a if j % 2 == 0 else hn_b
                        pv = pvs[j]
                        nc.vector.tensor_mul(hn[0:96, :], pv[0:96, :], rb)

                    def tail_b(j):
                        js = slice(j * NCH, (j + 1) * NCH)
                        hn = hn_a if j % 2 == 0 else hn_b
                        pj = MSC_tile([C, NCH], F32, name=f"pj{j}")
                        nc.tensor.matmul(pj, wb_p, hn, start=True, stop=True)
                        ot = OPO.tile([C, NCH], F32, tag="ot", name=f"ot{j}")
                        nc.vector.scalar_tensor_tensor(out=ot,
                                                       in0=x_sb[:, js],
                                                       scalar=res_c, in1=pj,
                                                       op0=OP.mult,
                                                       op1=OP.add)
                        nc.sync.dma_start(out=d_out.ap()[:, js], in_=ot)

                    # PE warm-up burst right before the attention phase
                    for _w in range(20):
                        ring_filler(SPOOLS[_w % 3])
                    for j in range(NJ):
                        new_pv(j)
                        for gs in range(0, 32, 2):
                            attn_super(j, gs)
                            # tails AFTER the super so this super's exp never
                            # queues behind a tail op in an engine FIFO
                            if j > 0 and gs == 4:
                                denom_tail(j - 1)
                            if j > 0 and gs == 8:
                                tail_a(j - 1)
                            if j > 0 and gs == 12:
                                tail_b(j - 1)
                        attn_end(j)
                    denom_tail(NJ - 1)
                    tail_a(NJ - 1)
                    tail_b(NJ - 1)
    nc.compile()
    return nc


def _prep_inputs(x, norm_w, norm_b, qkv_w, qkv_b, proj_w, proj_b):
    xf = np.ascontiguousarray(x.reshape(B, C, N), dtype=np.float32)
    qkv_wf = (qkv_w * norm_w[None, :]).astype(np.float32)
    qkv_bf = (qkv_b + qkv_w @ norm_b).astype(np.float32)
    in_maps = []
    for core in range(8):
        b, hp = core // 2, core % 2
        h0, h1 = 2 * hp, 2 * hp + 1
        qrows = list(range(h0 * DH, h0 * DH + DH)) + \
            list(range(h1 * DH, h1 * DH + DH))
        krows = [C + r for r in qrows]
        vrows = [2 * C + r for r in qrows]
        qrows2 = qrows + qrows
        krows2 = krows + krows
        wqq = qkv_wf[qrows2, :].T.copy()
        wkk = qkv_wf[krows2, :].T.copy()
        wv = qkv_wf[vrows, :].T.copy()
        bqq = qkv_bf[qrows2].reshape(128, 1).copy()
        bkk = qkv_bf[krows2].reshape(128, 1).copy()
        bv = np.broadcast_to(qkv_bf[vrows].reshape(1, 1, 64),
                             (C, 4, 64)).copy()
        h0rows = qrows[0:DH]
        h1rows = qrows[DH:2 * DH]
        pw = np.zeros((97, C), np.float32)
        pw[0:32, :] = proj_w[:, h0rows].T
        pw[64:96, :] = proj_w[:, h1rows].T
        if hp == 0:
            pw[96, :] = proj_b
        res = np.full((C, 1), 1.0 if hp == 0 else 0.0, np.float32)
        in_maps.append({
            "x": np.ascontiguousarray(xf[b]), "wqq": wqq, "wkk": wkk,
            "wv": wv, "bqq": bqq, "bkk": bkk, "bv": bv, "pw": pw, "res": res,
        })
    return in_maps


_NC_CACHE = None
TRACE = False
LAST_RESULTS = None


def kernel(x, norm_w, norm_b, qkv_w, qkv_b, proj_w, proj_b, **extra):
    global _NC_CACHE, LAST_RESULTS
    x = np.asarray(x, dtype=np.float32)
    in_maps = _prep_inputs(x, np.asarray(norm_w), np.asarray(norm_b),
                           np.asarray(qkv_w), np.asarray(qkv_b),
                           np.asarray(proj_w), np.asarray(proj_b))
    if _NC_CACHE is None:
        _NC_CACHE = build_nc()
    res = run_bass_kernel_spmd(_NC_CACHE, in_maps, core_ids=list(range(8)),
                               trace=TRACE)
    LAST_RESULTS = res
    parts = [res.results[i]["out"] for i in range(8)]
    out = np.empty((B, C, N), np.float32)
    for b in range(B):
        out[b] = parts[2 * b] + parts[2 * b + 1]
    return out.reshape(B, C, H, W)


if __name__ == "__main__":
    rng = np.random.default_rng(0)
    x = rng.standard_normal((B, C, H, W)).astype(np.float32)
    nw = np.ones(C, np.float32)
    nb = np.zeros(C, np.float32)
    qw = (rng.standard_normal((3 * C, C)) / np.sqrt(C)).astype(np.float32)
    qb = np.zeros(3 * C, np.float32)
    pw = (rng.standard_normal((C, C)) / np.sqrt(C)).astype(np.float32)
    pb = np.zeros(C, np.float32)
    got = kernel(x, nw, nb, qw, qb, pw, pb)
    print("kernel ran, shape", got.shape)
